# revision 12
# baseline (speedup 1.0000x reference)
"""Trainium2 Bass kernel for nn_Branch_74560632258692 (PCT-style point cloud net).

Sharding: pure data parallel over batch B=8 -> one cloud per NeuronCore (8 cores).

Host side (numpy, pure functions of the raw input / weights):
  - FPS + kNN index tensors (geometry only, derived from x[..., :3]).
  - BatchNorm folding into conv weights, weight transposes/chunking.
Device side (one NEFF, per core): stem convs, neighbor feature gathers
(dma_gather), both local MLPs with max-pool-over-K, 4 offset-attention
layers, final 1280->1024 conv + leaky relu + global max pool.

Key algebraic tricks (exact):
  - gather commutes with the per-point linear map: gather(W @ pts) instead of
    W @ gather(pts), so the expensive transpose happens once on [4096, C]
    instead of on [16384, C].
  - relu(h1) = max(Gt, -T) + T with T the per-centroid affine term; the +T
    part is folded into layer 2 as T2 = W2 @ T, and since T2 is constant
    over the K neighbor axis, max-pool commutes: pool(W2@M + T2) =
    pool(W2@M) + T2, applied after the pool on [C, S] instead of [C, S*K].
  - leaky_relu and relu commute with max-pool (monotone), so the final
    activation runs on pooled [1024, 1] columns.
Matmuls run in float32r (~1.6e-4 rel err, full PE rate); selection logic
(FPS/kNN) is host-fp32/fp64 so no discrete divergence.
"""

import numpy as np
from contextlib import ExitStack

_CACHE = {}


# ======================================================================
# Host: geometry (FPS + kNN)
# ======================================================================

def _fps(xyz, npoint):
    """Farthest point sampling, deterministic start at 0. xyz [B, N, 3] f32."""
    B, N, _ = xyz.shape
    dist = np.full((B, N), 1e10, np.float32)
    far = np.zeros(B, np.int64)
    idx = np.zeros((B, npoint), np.int64)
    ar = np.arange(B)
    for t in range(npoint):
        idx[:, t] = far
        c = xyz[ar, far][:, None, :]
        d = ((xyz - c) ** 2).sum(-1)
        np.minimum(dist, d, out=dist)
        far = dist.argmax(-1)
    return idx


def _knn(q, pts, k):
    """Indices of the k nearest pts for each q row. q [B,S,3], pts [B,N,3]."""
    q64 = q.astype(np.float64)
    p64 = pts.astype(np.float64)
    d2 = ((q64 ** 2).sum(-1)[..., None]
          - 2.0 * np.einsum('bsc,bnc->bsn', q64, p64)
          + (p64 ** 2).sum(-1)[:, None, :])
    return np.argpartition(d2, k - 1, axis=-1)[..., :k]


def _wrap_idx(flat, chunk):
    """dma_gather index layout: per gather-chunk, idx i lives at
    [i % 16, i // 16]; 16-row pattern replicated to 128 partitions."""
    flat = np.asarray(flat, np.int64)
    cols = []
    for c0 in range(0, len(flat), chunk):
        seg = flat[c0:c0 + chunk]
        cols.append(seg.reshape(len(seg) // 16, 16).T)
    w16 = np.concatenate(cols, axis=1)
    return np.ascontiguousarray(np.tile(w16, (8, 1)).astype(np.int16))


def _host_geometry(x_b):
    """All index tensors for one cloud. x_b [4096, 7] f32."""
    xyz = x_b[None, :, :3]
    fps1 = _fps(xyz, 512)[0]
    new_xyz = xyz[0][fps1]
    fps2 = _fps(new_xyz[None], 256)[0]
    knn1 = _knn(new_xyz[None], xyz, 32)[0]          # [512, 32] into 4096
    knn2 = _knn(new_xyz[fps2][None], new_xyz[None], 32)[0]  # [256, 32] into 512
    return {
        'idx_np1': _wrap_idx(fps1, 512),             # [128, 32]
        'idx_knn1': _wrap_idx(knn1.reshape(-1), 2048),   # [128, 1024]
        'idx_f2': _wrap_idx(fps2, 256),              # [128, 16]
        'idx_knn2': _wrap_idx(knn2.reshape(-1), 2048),   # [128, 512]
    }


# ======================================================================
# Host: weight folding
# ======================================================================

def _chunkT(w, kc):
    """lhsT chunks: w [O, C] (+scale folded) -> list of kc [128, O] blocks of w.T."""
    wT = np.ascontiguousarray(w.T)  # [C, O]
    C = wT.shape[0]
    step = C // kc
    return np.stack([wT[i * step:(i + 1) * step] for i in range(kc)])


def _colchunks(v, n):
    """bias [n*128] -> [128, n] with b_dev[p, o] = v[o*128 + p]."""
    return np.ascontiguousarray(np.asarray(v, np.float32).reshape(n, 128).T)


def _prep_weights(params):
    f = lambda a: np.asarray(a, np.float32)
    P = {}
    # stem
    P['stem_w1T'] = f(params['conv1_w'] * params['bn1_s'][:, None]).T.copy()  # [7, 64]
    P['stem_b1'] = f(params['bn1_b']).reshape(64, 1).copy()
    P['stem_w2T'] = f(params['conv2_w'] * params['bn2_s'][:, None]).T.copy()  # [64, 64]
    P['stem_b2'] = f(params['bn2_b']).reshape(64, 1).copy()
    # l0 (D=64 in, 128 feat, 128 hidden/out)
    l0 = params['l0']
    w1 = f(l0['w1']) * f(l0['s1'])[:, None]          # [128, 128] scaled
    P['l0_Arhs'] = np.ascontiguousarray(w1[:, :64].T)        # [64, 128]
    P['l0_BmT'] = np.ascontiguousarray((w1[:, 64:] - w1[:, :64]).T)  # [64, 128]
    P['l0_b1'] = f(l0['b1']).reshape(128, 1).copy()
    w2 = f(l0['w2']) * f(l0['s2'])[:, None]          # [128, 128]
    P['l0_W2sT'] = np.ascontiguousarray(w2.T)                # [128, 128]
    P['l0_negW2sT'] = np.ascontiguousarray(-w2.T)
    P['l0_b2'] = f(l0['b2']).reshape(128, 1).copy()
    # l1 (D=128 in, 256 feat, 256 hidden/out)
    l1 = params['l1']
    w1 = f(l1['w1']) * f(l1['s1'])[:, None]          # [256, 256]
    P['l1_Arhs'] = np.ascontiguousarray(w1[:, :128].T)       # [128, 256]
    P['l1_BmT'] = np.ascontiguousarray((w1[:, 128:] - w1[:, :128]).T)  # [128, 256]
    P['l1_b1'] = _colchunks(l1['b1'], 2)
    w2 = f(l1['w2']) * f(l1['s2'])[:, None]
    P['l1_W2sT'] = _chunkT(w2, 2)                    # [2, 128, 256]
    P['l1_negW2sT'] = _chunkT(-w2, 2)
    P['l1_b2'] = _colchunks(l1['b2'], 2)
    # pt convs
    P['ptc1T'] = _chunkT(f(params['pt_c1_w']) * f(params['pt_bn1_s'])[:, None], 2)
    P['ptb1'] = _colchunks(params['pt_bn1_b'], 2)
    P['ptc2T'] = _chunkT(f(params['pt_c2_w']) * f(params['pt_bn2_s'])[:, None], 2)
    P['ptb2'] = _colchunks(params['pt_bn2_b'], 2)
    # SA layers
    for i, sp in enumerate(params['sa']):
        P[f'sa{i}_wqkT'] = _chunkT(f(sp['wqk']), 2)          # [2, 128, 64]
        P[f'sa{i}_wvT'] = _chunkT(f(sp['wv']), 2)            # [2, 128, 256]
        P[f'sa{i}_bvrow'] = f(sp['bv']).reshape(1, 256).copy()
        wts = f(sp['wt']) * f(sp['s'])[:, None]
        P[f'sa{i}_wtT'] = _chunkT(wts, 2)                    # [2, 128, 256]
        P[f'sa{i}_bfin'] = _colchunks(f(sp['s']) * f(sp['bt']) + f(sp['b']), 2)
    # final conv
    cfw = f(params['cf_w']) * f(params['cf_s'])[:, None]     # [1024, 1280]
    cfT = np.ascontiguousarray(cfw.T)                        # [1280, 1024]
    P['cfT'] = np.ascontiguousarray(
        cfT.reshape(10, 128, 1024).transpose(1, 0, 2).reshape(128, 10240))
    P['cfb'] = _colchunks(params['cf_b'], 8)
    # constants
    P['id128'] = np.eye(128, dtype=np.float32)
    P['id64'] = np.eye(64, dtype=np.float32)
    P['ones1'] = np.ones((1, 128), np.float32)
    P['ones128col'] = np.ones((128, 4), np.float32)
    return P


# ======================================================================
# Device kernel
# ======================================================================

def build_nc(phase='full'):
    import concourse.bacc as bacc
    import concourse.bass as bass
    import concourse.mybir as mybir
    import concourse.tile as tile

    F32 = mybir.dt.float32
    F32R = mybir.dt.float32r
    I16 = mybir.dt.int16
    MAX = mybir.AluOpType.max
    ADD = mybir.AluOpType.add
    SUB = mybir.AluOpType.subtract
    MULT = mybir.AluOpType.mult
    AX = mybir.AxisListType.X
    AF = mybir.ActivationFunctionType

    nc = bacc.Bacc(None, target_bir_lowering=False)

    def par(name, shape, dt=F32R):
        return nc.declare_dram_parameter(name, list(shape), dt, isOutput=False)

    # ---- inputs
    x = par('x', [4096, 7])
    cfT_d = par('cfT', [128, 10240])
    idx_np1 = par('idx_np1', [128, 32], I16)
    idx_knn1 = par('idx_knn1', [128, 1024], I16)
    idx_f2 = par('idx_f2', [128, 16], I16)
    idx_knn2 = par('idx_knn2', [128, 512], I16)

    W = {}
    wspec = {
        'stem_w1T': ([7, 64], F32R), 'stem_b1': ([64, 1], F32),
        'stem_w2T': ([64, 64], F32R), 'stem_b2': ([64, 1], F32),
        'l0_Arhs': ([64, 128], F32R), 'l0_BmT': ([64, 128], F32R),
        'l0_b1': ([128, 1], F32), 'l0_W2sT': ([128, 128], F32R),
        'l0_negW2sT': ([128, 128], F32R), 'l0_b2': ([128, 1], F32),
        'l1_Arhs': ([128, 256], F32R), 'l1_BmT': ([128, 256], F32R),
        'l1_b1': ([128, 2], F32), 'l1_W2sT': ([2, 128, 256], F32R),
        'l1_negW2sT': ([2, 128, 256], F32R), 'l1_b2': ([128, 2], F32),
        'ptc1T': ([2, 128, 256], F32R), 'ptb1': ([128, 2], F32),
        'ptc2T': ([2, 128, 256], F32R), 'ptb2': ([128, 2], F32),
        'cfb': ([128, 8], F32),
        'id128': ([128, 128], F32R), 'id64': ([64, 64], F32R),
        'ones1': ([1, 128], F32R), 'ones128col': ([128, 4], F32R),
    }
    for i in range(4):
        wspec[f'sa{i}_wqkT'] = ([2, 128, 64], F32R)
        wspec[f'sa{i}_wvT'] = ([2, 128, 256], F32R)
        wspec[f'sa{i}_bvrow'] = ([1, 256], F32R)
        wspec[f'sa{i}_wtT'] = ([2, 128, 256], F32R)
        wspec[f'sa{i}_bfin'] = ([128, 2], F32)
    for name, (shape, dt) in wspec.items():
        W[name] = par(name, shape, dt)

    out = nc.declare_dram_parameter('out', [1024], F32, isOutput=True)

    # ---- internal DRAM staging
    A_pm_dram = nc.dram_tensor('A_pm', [4096, 128], F32R)
    pts_pm_dram = nc.dram_tensor('pts_pm', [4096, 64], F32R)
    f0_pm_dram = nc.dram_tensor('f0_pm', [512, 128], F32R)
    A2_pm_dram = nc.dram_tensor('A2_pm', [512, 256], F32R)

    class _PhaseDone(Exception):
        pass

    with tile.TileContext(nc) as tc, ExitStack() as ctx:
      try:
        wp = ctx.enter_context(tc.tile_pool(name='w', bufs=1))
        sp = ctx.enter_context(tc.tile_pool(name='stage', bufs=1))
        gp = ctx.enter_context(tc.tile_pool(name='gather', bufs=2))
        mp = ctx.enter_context(tc.tile_pool(name='m', bufs=3))
        ep = ctx.enter_context(tc.tile_pool(name='evict', bufs=3))
        pp = ctx.enter_context(tc.tile_pool(name='ps', bufs=6, space='PSUM'))

        def psum(p_, f_, dt=F32):
            return pp.tile([p_, f_], dt, tag='pb', name='pb')

        # load weights/constants
        ws = {}
        for name, (shape, dt) in wspec.items():
            if len(shape) == 3:
                k_, p_, f_ = shape
                t = wp.tile([p_, k_, f_], dt, tag=f'w_{name}')
                for kk in range(k_):
                    nc.sync.dma_start(t[:, kk, :], W[name][kk])
            else:
                t = wp.tile(list(shape), dt, tag=f'w_{name}')
                nc.sync.dma_start(t[:], W[name][:])
            ws[name] = t
        ix1 = wp.tile([128, 32], I16, tag='ix1')
        nc.sync.dma_start(ix1[:], idx_np1[:])
        ixk1 = wp.tile([128, 1024], I16, tag='ixk1')
        nc.sync.dma_start(ixk1[:], idx_knn1[:])
        ixf2 = wp.tile([128, 16], I16, tag='ixf2')
        nc.sync.dma_start(ixf2[:], idx_f2[:])
        ixk2 = wp.tile([128, 512], I16, tag='ixk2')
        nc.sync.dma_start(ixk2[:], idx_knn2[:])


        def _dbg_out(ap2d, f_):
            # write a [128, f] (or [p, f]) debug view into out[1024]
            nc.sync.dma_start(
                out[:].rearrange('(p f) -> p f', f=f_)[:ap2d.shape[0], :],
                ap2d.bitcast(F32))
        # ---------------- stem ----------------
        x_sb = gp.tile([7, 4096], F32R, tag='g', name='x_sb')
        nc.sync.dma_start(x_sb[:], x[:].rearrange('n c -> c n'))
        h1 = gp.tile([64, 4096], F32R, tag='g', name='h1')
        for c in range(8):
            ps = psum(64, 512)
            nc.tensor.matmul(ps[:], ws['stem_w1T'][:], x_sb[:, c * 512:(c + 1) * 512],
                             start=True, stop=True)
            nc.scalar.activation(h1[:, c * 512:(c + 1) * 512], ps[:], AF.Relu,
                                 bias=ws['stem_b1'][:])
        pts = gp.tile([64, 4096], F32R, tag='g', name='pts')
        for c in range(8):
            ps = psum(64, 512)
            nc.tensor.matmul(ps[:], ws['stem_w2T'][:], h1[:, c * 512:(c + 1) * 512],
                             start=True, stop=True)
            nc.scalar.activation(pts[:, c * 512:(c + 1) * 512], ps[:], AF.Relu,
                                 bias=ws['stem_b2'][:])

        if phase == 'stem':
            _dbg_out(pts[:, 0:16], 16)
            raise _PhaseDone()
        # ---------------- premultiplied tables (A = W1a_s @ pts, point-major) ----
        for c in range(32):
            ps = psum(128, 128)
            nc.tensor.matmul(ps[:], pts[:, c * 128:(c + 1) * 128], ws['l0_Arhs'][:],
                             start=True, stop=True)
            ev = ep.tile([128, 128], F32R, tag='ev')
            nc.vector.tensor_copy(ev[:], ps[:])
            nc.sync.dma_start(A_pm_dram[c * 128:(c + 1) * 128, :], ev[:])
        for c in range(32):
            ps = psum(128, 64)
            nc.tensor.matmul(ps[:], pts[:, c * 128:(c + 1) * 128], ws['id64'][:],
                             start=True, stop=True)
            ev = ep.tile([128, 64], F32R, tag='evp')
            nc.vector.tensor_copy(ev[:], ps[:])
            nc.sync.dma_start(pts_pm_dram[c * 128:(c + 1) * 128, :], ev[:])

        if phase == 'premult':
            pmb = sp.tile([8, 128], F32R, tag='pmb')
            nc.sync.dma_start(pmb[:], A_pm_dram[0:8, :])
            _dbg_out(pmb[:], 128)
            raise _PhaseDone()
        # ---------------- T-term, stage 1 ----------------
        np1g = sp.tile([128, 4, 64], F32R, tag='np1g')
        nc.gpsimd.dma_gather(np1g[:], pts_pm_dram[:], ix1[:], 512, 512, 64,
                             elem_step=64)
        psn = psum(64, 512, F32R)
        for b in range(4):
            nc.tensor.transpose(psn[:, b * 128:(b + 1) * 128], np1g[:, b, :],
                                ws['id128'][:])
        np_och = sp.tile([64, 512], F32R, tag='np_och')
        nc.vector.tensor_copy(np_och[:], psn[:])
        psT = psum(128, 512)
        nc.tensor.matmul(psT[:], ws['l0_BmT'][:], np_och[:], start=True, stop=True)
        negT = sp.tile([128, 512], F32R, tag='negT')
        nc.vector.tensor_scalar(negT[:], psT[:], ws['l0_b1'][:], -1.0, ADD, MULT)
        psT2 = psum(128, 512)
        nc.tensor.matmul(psT2[:], ws['l0_negW2sT'][:], negT[:], start=True, stop=True)
        T2 = sp.tile([128, 512], F32, tag='T2')
        nc.vector.tensor_copy(T2[:], psT2[:])

        if phase == 't1':
            _dbg_out(negT[:, 0:8], 8)
            raise _PhaseDone()
        # ---------------- stage 1 main loop ----------------
        pooled1 = sp.tile([128, 512], F32, tag='pooled1')
        for c in range(8):
            G = gp.tile([128, 16, 128], F32R, tag='g', name='G')
            nc.gpsimd.dma_gather(G[:], A_pm_dram[:], ixk1[:, c * 128:(c + 1) * 128],
                                 2048, 2048, 128, elem_step=128,
                                 single_packet=False)
            for q in range(4):
                s0 = c * 64 + q * 16  # first s-index of this 512-g group
                pg = psum(128, 512, F32R)
                for j in range(4):
                    nc.tensor.transpose(pg[:, j * 128:(j + 1) * 128],
                                        G[:, q * 4 + j, :], ws['id128'][:])
                M = mp.tile([128, 512], F32R, tag='m')
                nc.vector.tensor_tensor(
                    out=M[:].rearrange('p (s k) -> p s k', k=32),
                    in0=pg[:].rearrange('p (s k) -> p s k', k=32),
                    in1=negT[:, s0:s0 + 16].unsqueeze(2).broadcast_to([128, 16, 32]),
                    op=MAX)
                pl = psum(128, 512)
                nc.tensor.matmul(pl[:], ws['l0_W2sT'][:], M[:], start=True, stop=True)
                nc.vector.tensor_reduce(
                    pooled1[:, s0:s0 + 16],
                    pl[:].rearrange('p (s k) -> p s k', k=32), AX, MAX)
        f0 = sp.tile([128, 512], F32R, tag='f0')
        ftmp = sp.tile([128, 512], F32, tag='ftmp')
        nc.vector.tensor_tensor(out=ftmp[:], in0=pooled1[:], in1=T2[:], op=ADD)
        nc.scalar.activation(f0[:], ftmp[:], AF.Relu, bias=ws['l0_b2'][:])

        if phase == 'stage1':
            _dbg_out(f0[:, 0:8], 8)
            raise _PhaseDone()
        # ---------------- stage 2 prep ----------------
        for t in range(4):
            ps = psum(128, 128)
            nc.tensor.matmul(ps[:], f0[:, t * 128:(t + 1) * 128], ws['id128'][:],
                             start=True, stop=True)
            ev = ep.tile([128, 128], F32R, tag='ev')
            nc.vector.tensor_copy(ev[:], ps[:])
            nc.sync.dma_start(f0_pm_dram[t * 128:(t + 1) * 128, :], ev[:])
        for t in range(4):
            ps = psum(128, 256)
            nc.tensor.matmul(ps[:], f0[:, t * 128:(t + 1) * 128], ws['l1_Arhs'][:],
                             start=True, stop=True)
            ev = ep.tile([128, 256], F32R, tag='ev2')
            nc.vector.tensor_copy(ev[:], ps[:])
            nc.sync.dma_start(A2_pm_dram[t * 128:(t + 1) * 128, :], ev[:])

        np2g = sp.tile([128, 2, 128], F32R, tag='np2g')
        nc.gpsimd.dma_gather(np2g[:], f0_pm_dram[:], ixf2[:], 256, 256, 128,
                             elem_step=128)
        psn2 = psum(128, 256, F32R)
        for b in range(2):
            nc.tensor.transpose(psn2[:, b * 128:(b + 1) * 128], np2g[:, b, :],
                                ws['id128'][:])
        np2_och = sp.tile([128, 256], F32R, tag='np2_och')
        nc.vector.tensor_copy(np2_och[:], psn2[:])
        negT1 = []
        for o in range(2):
            ps = psum(128, 256)
            nc.tensor.matmul(ps[:], ws['l1_BmT'][:, o * 128:(o + 1) * 128],
                             np2_och[:], start=True, stop=True)
            nt = sp.tile([128, 256], F32R, tag=f'negT1_{o}')
            nc.vector.tensor_scalar(nt[:], ps[:], ws['l1_b1'][:, o:o + 1], -1.0,
                                    ADD, MULT)
            negT1.append(nt)
        T2l1 = []
        for o in range(2):
            ps = psum(128, 256)
            nc.tensor.matmul(ps[:], ws['l1_negW2sT'][:, 0, o * 128:(o + 1) * 128],
                             negT1[0][:], start=True, stop=False)
            nc.tensor.matmul(ps[:], ws['l1_negW2sT'][:, 1, o * 128:(o + 1) * 128],
                             negT1[1][:], start=False, stop=True)
            t2 = sp.tile([128, 256], F32, tag=f'T2l1_{o}')
            nc.vector.tensor_copy(t2[:], ps[:])
            T2l1.append(t2)

        if phase == 'stage2prep':
            _dbg_out(negT1[0][:, 0:8], 8)
            raise _PhaseDone()
        # ---------------- stage 2 main loop ----------------
        pooled2 = [sp.tile([128, 256], F32, tag=f'pooled2_{o}', name=f'pooled2_{o}') for o in range(2)]
        for c in range(4):
            G2 = gp.tile([128, 16, 256], F32R, tag='g', name='G2')
            nc.gpsimd.dma_gather(G2[:], A2_pm_dram[:], ixk2[:, c * 128:(c + 1) * 128],
                                 2048, 2048, 256, elem_step=256,
                                 single_packet=False)
            for q in range(4):
                t0 = c * 64 + q * 16
                Ms = []
                for k in range(2):
                    pg = psum(128, 512, F32R)
                    for j in range(4):
                        nc.tensor.transpose(
                            pg[:, j * 128:(j + 1) * 128],
                            G2[:, q * 4 + j, k * 128:(k + 1) * 128], ws['id128'][:])
                    M = mp.tile([128, 512], F32R, tag='m')
                    nc.vector.tensor_tensor(
                        out=M[:].rearrange('p (s k) -> p s k', k=32),
                        in0=pg[:].rearrange('p (s k) -> p s k', k=32),
                        in1=negT1[k][:, t0:t0 + 16].unsqueeze(2)
                            .broadcast_to([128, 16, 32]),
                        op=MAX)
                    Ms.append(M)
                for o in range(2):
                    pl = psum(128, 512)
                    nc.tensor.matmul(pl[:], ws['l1_W2sT'][:, 0, o * 128:(o + 1) * 128],
                                     Ms[0][:], start=True, stop=False)
                    nc.tensor.matmul(pl[:], ws['l1_W2sT'][:, 1, o * 128:(o + 1) * 128],
                                     Ms[1][:], start=False, stop=True)
                    nc.vector.tensor_reduce(
                        pooled2[o][:, t0:t0 + 16],
                        pl[:].rearrange('p (s k) -> p s k', k=32), AX, MAX)
        f1 = []
        for o in range(2):
            ft = sp.tile([128, 256], F32, tag=f'f1tmp_{o}')
            nc.vector.tensor_tensor(out=ft[:], in0=pooled2[o][:], in1=T2l1[o][:],
                                    op=ADD)
            f1o = sp.tile([128, 256], F32R, tag=f'f1_{o}')
            nc.scalar.activation(f1o[:], ft[:], AF.Relu, bias=ws['l1_b2'][:, o:o + 1])
            f1.append(f1o)

        if phase == 'stage2':
            _dbg_out(f1[0][:, 0:8], 8)
            raise _PhaseDone()
        # ---------------- pt convs ----------------
        def conv2x2(hin, wT, bias, outtag):
            outs = []
            for o in range(2):
                ps = psum(128, 256)
                nc.tensor.matmul(ps[:], wT[:, 0, o * 128:(o + 1) * 128], hin[0][:],
                                 start=True, stop=False)
                nc.tensor.matmul(ps[:], wT[:, 1, o * 128:(o + 1) * 128], hin[1][:],
                                 start=False, stop=True)
                ho = sp.tile([128, 256], F32R, tag=f'{outtag}_{o}')
                nc.scalar.activation(ho[:], ps[:], AF.Relu, bias=bias[:, o:o + 1])
                outs.append(ho)
            return outs

        hA = conv2x2(f1, ws['ptc1T'], ws['ptb1'], 'hA')
        h = conv2x2(hA, ws['ptc2T'], ws['ptb2'], 'hB')

        # ---------------- 4 offset-attention layers ----------------
        conc = []  # 10 [128, 256] k-chunks for the final conv, in order
        for i in range(4):
            wqkT, wvT = ws[f'sa{i}_wqkT'], ws[f'sa{i}_wvT']
            wtT, bfin = ws[f'sa{i}_wtT'], ws[f'sa{i}_bfin']
            # k/q: [64, 256]
            psk = psum(64, 256)
            nc.tensor.matmul(psk[:], wqkT[:, 0, :], h[0][:], start=True, stop=False)
            nc.tensor.matmul(psk[:], wqkT[:, 1, :], h[1][:], start=False, stop=True)
            kq = sp.tile([64, 256], F32R, tag='kq')
            nc.vector.tensor_copy(kq[:], psk[:])
            # energy + softmax per n-chunk
            attn = []
            rms = []
            for n in range(2):
                psE = psum(128, 256)
                nc.tensor.matmul(psE[:], kq[:, n * 128:(n + 1) * 128], kq[:],
                                 start=True, stop=True)
                mxn = sp.tile([128, 1], F32, tag='mxn')
                nc.vector.tensor_reduce(mxn[:], psE[:], AX, MAX, negate=True)
                aun = sp.tile([128, 256], F32, tag='aun')
                rs = sp.tile([128, 1], F32, tag='rs')
                nc.scalar.activation(aun[:], psE[:], AF.Exp, bias=mxn[:],
                                     accum_out=rs[:])
                rr = sp.tile([128, 1], F32, tag='rr')
                nc.vector.reciprocal(rr[:], rs[:])
                at = sp.tile([128, 256], F32R, tag=f'attn_{n}')
                nc.vector.tensor_scalar(at[:], aun[:], rr[:], None, MULT)
                attn.append(at)
            # column normalizer r = 1/(1e-9 + colsum), replicated to all rows
            rrow = sp.tile([1, 256], F32R, tag='rrow')
            for m in range(2):
                psc = psum(128, 4)
                nc.tensor.matmul(psc[:], attn[0][:, m * 128:(m + 1) * 128],
                                 ws['ones128col'][:], start=True, stop=False)
                nc.tensor.matmul(psc[:], attn[1][:, m * 128:(m + 1) * 128],
                                 ws['ones128col'][:], start=False, stop=True)
                cadd = sp.tile([128, 1], F32, tag='cadd')
                nc.vector.tensor_scalar(cadd[:], psc[:, 0:1], 1e-9, None, ADD)
                rm = sp.tile([128, 1], F32R, tag='rm')
                with nc.allow_low_precision('f32r bits == f32 bits here'):
                    nc.vector.reciprocal(rm[:], cadd[:])
                rms.append(rm)
                psr = psum(1, 128, F32R)
                nc.tensor.transpose(psr[:], rm[:], ws['id128'][:])
                nc.vector.tensor_copy(rrow[:, m * 128:(m + 1) * 128], psr[:])
            psrep = psum(128, 256)
            nc.tensor.matmul(psrep[:], ws['ones1'][:], rrow[:], start=True, stop=True)
            attn2 = []
            for n in range(2):
                a2 = sp.tile([128, 256], F32R, tag=f'attn2_{n}')
                nc.vector.tensor_tensor(out=a2[:], in0=attn[n][:], in1=psrep[:],
                                        op=MULT)
                attn2.append(a2)
            # vT [n, c] with bias seed
            vT = []
            for n in range(2):
                ps = psum(128, 256)
                nc.tensor.matmul(ps[:], ws['ones1'][:], ws[f'sa{i}_bvrow'][:],
                                 start=True, stop=False)
                nc.tensor.matmul(ps[:], h[0][:, n * 128:(n + 1) * 128], wvT[:, 0, :],
                                 start=False, stop=False)
                nc.tensor.matmul(ps[:], h[1][:, n * 128:(n + 1) * 128], wvT[:, 1, :],
                                 start=False, stop=True)
                vt = sp.tile([128, 256], F32R, tag=f'vT_{n}')
                nc.vector.tensor_copy(vt[:], ps[:])
                vT.append(vt)
            # x_r and d = h - x_r
            d = []
            for cc in range(2):
                ps = psum(128, 256)
                nc.tensor.matmul(ps[:], vT[0][:, cc * 128:(cc + 1) * 128],
                                 attn2[0][:], start=True, stop=False)
                nc.tensor.matmul(ps[:], vT[1][:, cc * 128:(cc + 1) * 128],
                                 attn2[1][:], start=False, stop=True)
                dc = sp.tile([128, 256], F32R, tag=f'd_{cc}')
                nc.vector.tensor_tensor(out=dc[:], in0=h[cc][:], in1=ps[:], op=SUB)
                d.append(dc)
            # t = relu(wts @ d + bfin); h = h + t
            hn = []
            for o in range(2):
                ps = psum(128, 256)
                nc.tensor.matmul(ps[:], wtT[:, 0, o * 128:(o + 1) * 128], d[0][:],
                                 start=True, stop=False)
                nc.tensor.matmul(ps[:], wtT[:, 1, o * 128:(o + 1) * 128], d[1][:],
                                 start=False, stop=True)
                xr2 = sp.tile([128, 256], F32, tag='xr2')
                nc.scalar.activation(xr2[:], ps[:], AF.Relu, bias=bfin[:, o:o + 1])
                hno = sp.tile([128, 256], F32R, tag=f'sa{i}o{o}')
                nc.vector.tensor_tensor(out=hno[:], in0=h[o][:], in1=xr2[:], op=ADD)
                hn.append(hno)
            h = hn
            conc.extend(hn)
        conc.extend(f1)

        if phase == 'att':
            _dbg_out(h[0][:, 0:8], 8)
            raise _PhaseDone()
        # ---------------- final conv + leaky relu + global max ----------------
        outsb = sp.tile([128, 8], F32, tag='outsb')
        for m in range(8):
            ps = psum(128, 256)
            for k in range(10):
                cfk = ep.tile([128, 128], F32R, tag='cfk', name='cfk')
                nc.sync.dma_start(
                    cfk[:], cfT_d[:, k * 1024 + m * 128:k * 1024 + (m + 1) * 128])
                nc.tensor.matmul(ps[:], cfk[:],
                                 conc[k][:], start=(k == 0), stop=(k == 9))
            mx = sp.tile([128, 1], F32, tag='mx')
            nc.vector.tensor_reduce(mx[:], ps[:], AX, MAX)
            y = sp.tile([128, 1], F32, tag='y')
            nc.vector.tensor_scalar(y[:], mx[:], ws['cfb'][:, m:m + 1], None, ADD)
            nc.vector.scalar_tensor_tensor(outsb[:, m:m + 1], y[:], 0.2, y[:],
                                           MULT, MAX)
        nc.sync.dma_start(out[:].rearrange('(m p) -> p m', p=128), outsb[:])

      except _PhaseDone:
        pass
    nc.compile()
    return nc


# ======================================================================
# Entry point
# ======================================================================

def kernel(x, params):
    from concourse.bass_utils import run_bass_kernel_spmd

    x = np.asarray(x, np.float32)
    B = x.shape[0]
    assert B == 8

    if 'nc' not in _CACHE:
        _CACHE['nc'] = build_nc()
    nc = _CACHE['nc']

    wmap = _prep_weights(params)
    in_maps = []
    for b in range(B):
        m = dict(wmap)
        m['x'] = np.ascontiguousarray(x[b])
        m.update(_host_geometry(x[b]))
        in_maps.append({k: np.ascontiguousarray(np.asarray(v)) for k, v in m.items()})

    res = run_bass_kernel_spmd(nc, in_maps, list(range(B)))
    return np.stack([res.results[b]['out'] for b in range(B)]).astype(np.float32)


# revision 13
# speedup vs baseline: 1.3134x; 1.3134x over previous
"""Trainium2 Bass kernel for nn_Branch_74560632258692 (PCT-style point cloud net).

Sharding: pure data parallel over batch B=8 -> one cloud per NeuronCore (8 cores).

Host side (numpy, pure functions of the raw input / weights):
  - FPS + kNN index tensors (geometry only, derived from x[..., :3]).
  - BatchNorm folding into conv weights, weight transposes/chunking.
Device side (one NEFF, per core): stem convs, neighbor feature gathers
(dma_gather), both local MLPs with max-pool-over-K, 4 offset-attention
layers, final 1280->1024 conv + leaky relu + global max pool.

Key algebraic tricks (exact):
  - gather commutes with the per-point linear map: gather(W @ pts) instead of
    W @ gather(pts), so the expensive transpose happens once on [4096, C]
    instead of on [16384, C].
  - relu(h1) = max(Gt, -T) + T with T the per-centroid affine term; the +T
    part is folded into layer 2 as T2 = W2 @ T, and since T2 is constant
    over the K neighbor axis, max-pool commutes: pool(W2@M + T2) =
    pool(W2@M) + T2, applied after the pool on [C, S] instead of [C, S*K].
  - leaky_relu and relu commute with max-pool (monotone), so the final
    activation runs on pooled [1024, 1] columns.
Matmuls run in float32r (~1.6e-4 rel err, full PE rate); selection logic
(FPS/kNN) is host-fp32/fp64 so no discrete divergence.
"""

import numpy as np
from contextlib import ExitStack

_CACHE = {}


# ======================================================================
# Host: geometry (FPS + kNN)
# ======================================================================

def _fps(xyz, npoint):
    """Farthest point sampling, deterministic start at 0. xyz [B, N, 3] f32."""
    B, N, _ = xyz.shape
    dist = np.full((B, N), 1e10, np.float32)
    far = np.zeros(B, np.int64)
    idx = np.zeros((B, npoint), np.int64)
    ar = np.arange(B)
    for t in range(npoint):
        idx[:, t] = far
        c = xyz[ar, far][:, None, :]
        d = ((xyz - c) ** 2).sum(-1)
        np.minimum(dist, d, out=dist)
        far = dist.argmax(-1)
    return idx


def _knn(q, pts, k):
    """Indices of the k nearest pts for each q row. q [B,S,3], pts [B,N,3]."""
    q64 = q.astype(np.float64)
    p64 = pts.astype(np.float64)
    d2 = ((q64 ** 2).sum(-1)[..., None]
          - 2.0 * np.einsum('bsc,bnc->bsn', q64, p64)
          + (p64 ** 2).sum(-1)[:, None, :])
    return np.argpartition(d2, k - 1, axis=-1)[..., :k]


def _wrap_idx(flat, chunk):
    """dma_gather index layout: per gather-chunk, idx i lives at
    [i % 16, i // 16]; 16-row pattern replicated to 128 partitions."""
    flat = np.asarray(flat, np.int64)
    cols = []
    for c0 in range(0, len(flat), chunk):
        seg = flat[c0:c0 + chunk]
        cols.append(seg.reshape(len(seg) // 16, 16).T)
    w16 = np.concatenate(cols, axis=1)
    return np.ascontiguousarray(np.tile(w16, (8, 1)).astype(np.int16))


def _host_geometry(x_b):
    """All index tensors for one cloud. x_b [4096, 7] f32."""
    xyz = x_b[None, :, :3]
    fps1 = _fps(xyz, 512)[0]
    new_xyz = xyz[0][fps1]
    fps2 = _fps(new_xyz[None], 256)[0]
    knn1 = _knn(new_xyz[None], xyz, 32)[0]          # [512, 32] into 4096
    knn2 = _knn(new_xyz[fps2][None], new_xyz[None], 32)[0]  # [256, 32] into 512
    return {
        'idx_np1': _wrap_idx(fps1, 512),             # [128, 32]
        'idx_knn1': _wrap_idx(knn1.reshape(-1), 2048),   # [128, 1024]
        'idx_f2': _wrap_idx(fps2, 256),              # [128, 16]
        'idx_knn2': _wrap_idx(knn2.reshape(-1), 2048),   # [128, 512]
    }


# ======================================================================
# Host: weight folding
# ======================================================================

def _chunkT(w, kc):
    """lhsT chunks: w [O, C] (+scale folded) -> list of kc [128, O] blocks of w.T."""
    wT = np.ascontiguousarray(w.T)  # [C, O]
    C = wT.shape[0]
    step = C // kc
    return np.stack([wT[i * step:(i + 1) * step] for i in range(kc)])


def _colchunks(v, n):
    """bias [n*128] -> [128, n] with b_dev[p, o] = v[o*128 + p]."""
    return np.ascontiguousarray(np.asarray(v, np.float32).reshape(n, 128).T)


def _prep_weights(params):
    f = lambda a: np.asarray(a, np.float32)
    P = {}
    # stem
    P['stem_w1T'] = f(params['conv1_w'] * params['bn1_s'][:, None]).T.copy()  # [7, 64]
    P['stem_b1'] = f(params['bn1_b']).reshape(64, 1).copy()
    P['stem_w2T'] = f(params['conv2_w'] * params['bn2_s'][:, None]).T.copy()  # [64, 64]
    P['stem_b2'] = f(params['bn2_b']).reshape(64, 1).copy()
    # l0 (D=64 in, 128 feat, 128 hidden/out)
    l0 = params['l0']
    w1 = f(l0['w1']) * f(l0['s1'])[:, None]          # [128, 128] scaled
    P['l0_Arhs'] = np.ascontiguousarray(w1[:, :64].T)        # [64, 128]
    P['l0_BmT'] = np.ascontiguousarray((w1[:, 64:] - w1[:, :64]).T)  # [64, 128]
    P['l0_b1'] = f(l0['b1']).reshape(128, 1).copy()
    w2 = f(l0['w2']) * f(l0['s2'])[:, None]          # [128, 128]
    P['l0_W2sT'] = np.ascontiguousarray(w2.T)                # [128, 128]
    P['l0_negW2sT'] = np.ascontiguousarray(-w2.T)
    P['l0_b2'] = f(l0['b2']).reshape(128, 1).copy()
    # l1 (D=128 in, 256 feat, 256 hidden/out)
    l1 = params['l1']
    w1 = f(l1['w1']) * f(l1['s1'])[:, None]          # [256, 256]
    P['l1_Arhs'] = np.ascontiguousarray(w1[:, :128].T)       # [128, 256]
    P['l1_BmT'] = np.ascontiguousarray((w1[:, 128:] - w1[:, :128]).T)  # [128, 256]
    P['l1_b1'] = _colchunks(l1['b1'], 2)
    w2 = f(l1['w2']) * f(l1['s2'])[:, None]
    P['l1_W2sT'] = _chunkT(w2, 2)                    # [2, 128, 256]
    P['l1_negW2sT'] = _chunkT(-w2, 2)
    P['l1_b2'] = _colchunks(l1['b2'], 2)
    # pt convs
    P['ptc1T'] = _chunkT(f(params['pt_c1_w']) * f(params['pt_bn1_s'])[:, None], 2)
    P['ptb1'] = _colchunks(params['pt_bn1_b'], 2)
    P['ptc2T'] = _chunkT(f(params['pt_c2_w']) * f(params['pt_bn2_s'])[:, None], 2)
    P['ptb2'] = _colchunks(params['pt_bn2_b'], 2)
    # SA layers
    for i, sp in enumerate(params['sa']):
        P[f'sa{i}_wqkT'] = _chunkT(f(sp['wqk']), 2)          # [2, 128, 64]
        P[f'sa{i}_wvT'] = _chunkT(f(sp['wv']), 2)            # [2, 128, 256]
        P[f'sa{i}_bvrow'] = f(sp['bv']).reshape(1, 256).copy()
        wts = f(sp['wt']) * f(sp['s'])[:, None]
        P[f'sa{i}_wtT'] = _chunkT(wts, 2)                    # [2, 128, 256]
        P[f'sa{i}_bfin'] = _colchunks(f(sp['s']) * f(sp['bt']) + f(sp['b']), 2)
    # final conv
    cfw = f(params['cf_w']) * f(params['cf_s'])[:, None]     # [1024, 1280]
    cfT = np.ascontiguousarray(cfw.T)                        # [1280, 1024]
    P['cfT'] = np.ascontiguousarray(
        cfT.reshape(10, 128, 1024).transpose(1, 0, 2).reshape(128, 10240))
    P['cfb'] = _colchunks(params['cf_b'], 8)
    # constants
    P['id128'] = np.eye(128, dtype=np.float32)
    P['id64'] = np.eye(64, dtype=np.float32)
    P['ones1'] = np.ones((1, 128), np.float32)
    P['ones128col'] = np.ones((128, 4), np.float32)
    return P


# ======================================================================
# Device kernel
# ======================================================================

def build_nc(phase='full'):
    import concourse.bacc as bacc
    import concourse.bass as bass
    import concourse.mybir as mybir
    import concourse.tile as tile

    F32 = mybir.dt.float32
    F32R = mybir.dt.float32r
    I16 = mybir.dt.int16
    MAX = mybir.AluOpType.max
    ADD = mybir.AluOpType.add
    SUB = mybir.AluOpType.subtract
    MULT = mybir.AluOpType.mult
    AX = mybir.AxisListType.X
    AF = mybir.ActivationFunctionType

    nc = bacc.Bacc(None, target_bir_lowering=False)

    def par(name, shape, dt=F32R):
        return nc.declare_dram_parameter(name, list(shape), dt, isOutput=False)

    # ---- inputs
    x = par('x', [4096, 7])
    idx_np1 = par('idx_np1', [128, 32], I16)
    idx_knn1 = par('idx_knn1', [128, 1024], I16)
    idx_f2 = par('idx_f2', [128, 16], I16)
    idx_knn2 = par('idx_knn2', [128, 512], I16)

    W = {}
    wspec = {
        'stem_w1T': ([7, 64], F32R), 'stem_b1': ([64, 1], F32),
        'stem_w2T': ([64, 64], F32R), 'stem_b2': ([64, 1], F32),
        'l0_Arhs': ([64, 128], F32R), 'l0_BmT': ([64, 128], F32R),
        'l0_b1': ([128, 1], F32), 'l0_W2sT': ([128, 128], F32R),
        'l0_negW2sT': ([128, 128], F32R), 'l0_b2': ([128, 1], F32),
        'l1_Arhs': ([128, 256], F32R), 'l1_BmT': ([128, 256], F32R),
        'l1_b1': ([128, 2], F32), 'l1_W2sT': ([2, 128, 256], F32R),
        'l1_negW2sT': ([2, 128, 256], F32R), 'l1_b2': ([128, 2], F32),
        'ptc1T': ([2, 128, 256], F32R), 'ptb1': ([128, 2], F32),
        'ptc2T': ([2, 128, 256], F32R), 'ptb2': ([128, 2], F32),
        'cfT': ([128, 10240], F32R), 'cfb': ([128, 8], F32),
        'id128': ([128, 128], F32R), 'id64': ([64, 64], F32R),
        'ones1': ([1, 128], F32R), 'ones128col': ([128, 4], F32R),
    }
    for i in range(4):
        wspec[f'sa{i}_wqkT'] = ([2, 128, 64], F32R)
        wspec[f'sa{i}_wvT'] = ([2, 128, 256], F32R)
        wspec[f'sa{i}_bvrow'] = ([1, 256], F32R)
        wspec[f'sa{i}_wtT'] = ([2, 128, 256], F32R)
        wspec[f'sa{i}_bfin'] = ([128, 2], F32)
    for name, (shape, dt) in wspec.items():
        W[name] = par(name, shape, dt)

    out = nc.declare_dram_parameter('out', [1024], F32, isOutput=True)

    # ---- internal DRAM staging
    A_pm_dram = nc.dram_tensor('A_pm', [4096, 128], F32R)
    pts_pm_dram = nc.dram_tensor('pts_pm', [4096, 64], F32R)
    f0_pm_dram = nc.dram_tensor('f0_pm', [512, 128], F32R)
    A2_pm_dram = nc.dram_tensor('A2_pm', [512, 256], F32R)

    class _PhaseDone(Exception):
        pass

    with tile.TileContext(nc) as tc, ExitStack() as ctx:
      try:
        wp = ctx.enter_context(tc.tile_pool(name='w', bufs=1))
        sp = ctx.enter_context(tc.tile_pool(name='stage', bufs=1))
        gp = ctx.enter_context(tc.tile_pool(name='gather', bufs=2))
        mp = ctx.enter_context(tc.tile_pool(name='m', bufs=3))
        ep = ctx.enter_context(tc.tile_pool(name='evict', bufs=3))
        pp = ctx.enter_context(tc.tile_pool(name='ps', bufs=8, space='PSUM'))

        def psum(p_, f_, dt=F32):
            return pp.tile([p_, f_], dt, tag='pb', name='pb')

        # load weights/constants
        ws = {}
        for name, (shape, dt) in wspec.items():
            if len(shape) == 3:
                k_, p_, f_ = shape
                t = wp.tile([p_, k_, f_], dt, tag=f'w_{name}')
                for kk in range(k_):
                    nc.sync.dma_start(t[:, kk, :], W[name][kk])
            else:
                t = wp.tile(list(shape), dt, tag=f'w_{name}')
                nc.sync.dma_start(t[:], W[name][:])
            ws[name] = t
        ix1 = wp.tile([128, 32], I16, tag='ix1')
        nc.sync.dma_start(ix1[:], idx_np1[:])
        ixk1 = wp.tile([128, 1024], I16, tag='ixk1')
        nc.sync.dma_start(ixk1[:], idx_knn1[:])
        ixf2 = wp.tile([128, 16], I16, tag='ixf2')
        nc.sync.dma_start(ixf2[:], idx_f2[:])
        ixk2 = wp.tile([128, 512], I16, tag='ixk2')
        nc.sync.dma_start(ixk2[:], idx_knn2[:])


        def _dbg_out(ap2d, f_):
            # write a [128, f] (or [p, f]) debug view into out[1024]
            nc.sync.dma_start(
                out[:].rearrange('(p f) -> p f', f=f_)[:ap2d.shape[0], :],
                ap2d.bitcast(F32))
        # ---------------- stem ----------------
        x_pm = gp.tile([128, 32, 7], F32R, tag='g', name='x_pm')
        nc.sync.dma_start(x_pm[:], x[:].rearrange('(a p) c -> p a c', p=128))
        x_sb = sp.tile([7, 4096], F32R, tag='x_sb')
        for c8 in range(8):
            psx = psum(7, 512, F32R)
            for j in range(4):
                nc.tensor.transpose(psx[:, j * 128:(j + 1) * 128],
                                    x_pm[:, c8 * 4 + j, :], ws['id128'][:])
            nc.vector.tensor_copy(x_sb[:, c8 * 512:(c8 + 1) * 512], psx[:])
        h1 = gp.tile([64, 4096], F32R, tag='g', name='h1')
        for c in range(8):
            ps = psum(64, 512)
            nc.tensor.matmul(ps[:], ws['stem_w1T'][:], x_sb[:, c * 512:(c + 1) * 512],
                             start=True, stop=True)
            nc.scalar.activation(h1[:, c * 512:(c + 1) * 512], ps[:], AF.Relu,
                                 bias=ws['stem_b1'][:])
        pts = gp.tile([64, 4096], F32R, tag='g', name='pts')
        for c in range(8):
            ps = psum(64, 512)
            nc.tensor.matmul(ps[:], ws['stem_w2T'][:], h1[:, c * 512:(c + 1) * 512],
                             start=True, stop=True)
            nc.scalar.activation(pts[:, c * 512:(c + 1) * 512], ps[:], AF.Relu,
                                 bias=ws['stem_b2'][:])

        if phase == 'stem':
            _dbg_out(pts[:, 0:16], 16)
            raise _PhaseDone()
        # ---------------- premultiplied tables (A = W1a_s @ pts, point-major) ----
        for c in range(32):
            ps = psum(128, 128)
            nc.tensor.matmul(ps[:], pts[:, c * 128:(c + 1) * 128], ws['l0_Arhs'][:],
                             start=True, stop=True)
            ev = ep.tile([128, 128], F32R, tag='ev')
            nc.vector.tensor_copy(ev[:], ps[:])
            nc.sync.dma_start(A_pm_dram[c * 128:(c + 1) * 128, :], ev[:])
        for c in range(32):
            ps = psum(128, 64)
            nc.tensor.matmul(ps[:], pts[:, c * 128:(c + 1) * 128], ws['id64'][:],
                             start=True, stop=True)
            ev = ep.tile([128, 64], F32R, tag='evp')
            nc.vector.tensor_copy(ev[:], ps[:])
            nc.sync.dma_start(pts_pm_dram[c * 128:(c + 1) * 128, :], ev[:])

        if phase == 'premult':
            pmb = sp.tile([8, 128], F32R, tag='pmb')
            nc.sync.dma_start(pmb[:], A_pm_dram[0:8, :])
            _dbg_out(pmb[:], 128)
            raise _PhaseDone()
        # ---------------- T-term, stage 1 ----------------
        np1g = sp.tile([128, 4, 64], F32R, tag='np1g')
        nc.gpsimd.dma_gather(np1g[:], pts_pm_dram[:], ix1[:], 512, 512, 64,
                             elem_step=64)
        psn = psum(64, 512, F32R)
        for b in range(4):
            nc.tensor.transpose(psn[:, b * 128:(b + 1) * 128], np1g[:, b, :],
                                ws['id128'][:])
        np_och = sp.tile([64, 512], F32R, tag='np_och')
        nc.vector.tensor_copy(np_och[:], psn[:])
        psT = psum(128, 512)
        nc.tensor.matmul(psT[:], ws['l0_BmT'][:], np_och[:], start=True, stop=True)
        negT = sp.tile([128, 512], F32R, tag='negT')
        nc.vector.tensor_scalar(negT[:], psT[:], ws['l0_b1'][:], -1.0, ADD, MULT)
        psT2 = psum(128, 512)
        nc.tensor.matmul(psT2[:], ws['l0_negW2sT'][:], negT[:], start=True, stop=True)
        T2 = sp.tile([128, 512], F32, tag='T2')
        nc.vector.tensor_copy(T2[:], psT2[:])

        if phase == 't1':
            _dbg_out(negT[:, 0:8], 8)
            raise _PhaseDone()
        # ---------------- stage 1 main loop ----------------
        pooled1 = sp.tile([128, 512], F32, tag='pooled1')
        for c in range(8):
            G = gp.tile([128, 16, 128], F32R, tag='g', name='G')
            nc.gpsimd.dma_gather(G[:], A_pm_dram[:], ixk1[:, c * 128:(c + 1) * 128],
                                 2048, 2048, 128, elem_step=128,
                                 single_packet=False)
            for q in range(4):
                s0 = c * 64 + q * 16  # first s-index of this 512-g group
                pg = psum(128, 512, F32R)
                for j in range(4):
                    nc.tensor.transpose(pg[:, j * 128:(j + 1) * 128],
                                        G[:, q * 4 + j, :], ws['id128'][:])
                M = mp.tile([128, 512], F32R, tag='m')
                nc.vector.tensor_tensor(
                    out=M[:].rearrange('p (s k) -> p s k', k=32),
                    in0=pg[:].rearrange('p (s k) -> p s k', k=32),
                    in1=negT[:, s0:s0 + 16].unsqueeze(2).broadcast_to([128, 16, 32]),
                    op=MAX)
                pl = psum(128, 512)
                nc.tensor.matmul(pl[:], ws['l0_W2sT'][:], M[:], start=True, stop=True)
                nc.vector.tensor_reduce(
                    pooled1[:, s0:s0 + 16],
                    pl[:].rearrange('p (s k) -> p s k', k=32), AX, MAX)
        f0 = sp.tile([128, 512], F32R, tag='f0')
        ftmp = sp.tile([128, 512], F32, tag='ftmp')
        nc.vector.tensor_tensor(out=ftmp[:], in0=pooled1[:], in1=T2[:], op=ADD)
        nc.scalar.activation(f0[:], ftmp[:], AF.Relu, bias=ws['l0_b2'][:])

        if phase == 'stage1':
            _dbg_out(f0[:, 0:8], 8)
            raise _PhaseDone()
        # ---------------- stage 2 prep ----------------
        for t in range(4):
            ps = psum(128, 128)
            nc.tensor.matmul(ps[:], f0[:, t * 128:(t + 1) * 128], ws['id128'][:],
                             start=True, stop=True)
            ev = ep.tile([128, 128], F32R, tag='ev')
            nc.vector.tensor_copy(ev[:], ps[:])
            nc.sync.dma_start(f0_pm_dram[t * 128:(t + 1) * 128, :], ev[:])
        for t in range(4):
            ps = psum(128, 256)
            nc.tensor.matmul(ps[:], f0[:, t * 128:(t + 1) * 128], ws['l1_Arhs'][:],
                             start=True, stop=True)
            ev = ep.tile([128, 256], F32R, tag='ev2')
            nc.vector.tensor_copy(ev[:], ps[:])
            nc.sync.dma_start(A2_pm_dram[t * 128:(t + 1) * 128, :], ev[:])

        np2g = sp.tile([128, 2, 128], F32R, tag='np2g')
        nc.gpsimd.dma_gather(np2g[:], f0_pm_dram[:], ixf2[:], 256, 256, 128,
                             elem_step=128)
        psn2 = psum(128, 256, F32R)
        for b in range(2):
            nc.tensor.transpose(psn2[:, b * 128:(b + 1) * 128], np2g[:, b, :],
                                ws['id128'][:])
        np2_och = sp.tile([128, 256], F32R, tag='np2_och')
        nc.vector.tensor_copy(np2_och[:], psn2[:])
        negT1 = []
        for o in range(2):
            ps = psum(128, 256)
            nc.tensor.matmul(ps[:], ws['l1_BmT'][:, o * 128:(o + 1) * 128],
                             np2_och[:], start=True, stop=True)
            nt = sp.tile([128, 256], F32R, tag=f'negT1_{o}')
            nc.vector.tensor_scalar(nt[:], ps[:], ws['l1_b1'][:, o:o + 1], -1.0,
                                    ADD, MULT)
            negT1.append(nt)
        T2l1 = []
        for o in range(2):
            ps = psum(128, 256)
            nc.tensor.matmul(ps[:], ws['l1_negW2sT'][:, 0, o * 128:(o + 1) * 128],
                             negT1[0][:], start=True, stop=False)
            nc.tensor.matmul(ps[:], ws['l1_negW2sT'][:, 1, o * 128:(o + 1) * 128],
                             negT1[1][:], start=False, stop=True)
            t2 = sp.tile([128, 256], F32, tag=f'T2l1_{o}')
            nc.vector.tensor_copy(t2[:], ps[:])
            T2l1.append(t2)

        if phase == 'stage2prep':
            _dbg_out(negT1[0][:, 0:8], 8)
            raise _PhaseDone()
        # ---------------- stage 2 main loop ----------------
        pooled2 = [sp.tile([128, 256], F32, tag=f'pooled2_{o}', name=f'pooled2_{o}') for o in range(2)]
        for c in range(4):
            G2 = gp.tile([128, 16, 256], F32R, tag='g', name='G2')
            nc.gpsimd.dma_gather(G2[:], A2_pm_dram[:], ixk2[:, c * 128:(c + 1) * 128],
                                 2048, 2048, 256, elem_step=256,
                                 single_packet=False)
            for q in range(4):
                t0 = c * 64 + q * 16
                Ms = []
                for k in range(2):
                    pg = psum(128, 512, F32R)
                    for j in range(4):
                        nc.tensor.transpose(
                            pg[:, j * 128:(j + 1) * 128],
                            G2[:, q * 4 + j, k * 128:(k + 1) * 128], ws['id128'][:])
                    M = mp.tile([128, 512], F32R, tag='m')
                    nc.vector.tensor_tensor(
                        out=M[:].rearrange('p (s k) -> p s k', k=32),
                        in0=pg[:].rearrange('p (s k) -> p s k', k=32),
                        in1=negT1[k][:, t0:t0 + 16].unsqueeze(2)
                            .broadcast_to([128, 16, 32]),
                        op=MAX)
                    Ms.append(M)
                for o in range(2):
                    pl = psum(128, 512)
                    nc.tensor.matmul(pl[:], ws['l1_W2sT'][:, 0, o * 128:(o + 1) * 128],
                                     Ms[0][:], start=True, stop=False)
                    nc.tensor.matmul(pl[:], ws['l1_W2sT'][:, 1, o * 128:(o + 1) * 128],
                                     Ms[1][:], start=False, stop=True)
                    nc.vector.tensor_reduce(
                        pooled2[o][:, t0:t0 + 16],
                        pl[:].rearrange('p (s k) -> p s k', k=32), AX, MAX)
        f1 = []
        for o in range(2):
            ft = sp.tile([128, 256], F32, tag=f'f1tmp_{o}')
            nc.vector.tensor_tensor(out=ft[:], in0=pooled2[o][:], in1=T2l1[o][:],
                                    op=ADD)
            f1o = sp.tile([128, 256], F32R, tag=f'f1_{o}')
            nc.scalar.activation(f1o[:], ft[:], AF.Relu, bias=ws['l1_b2'][:, o:o + 1])
            f1.append(f1o)

        if phase == 'stage2':
            _dbg_out(f1[0][:, 0:8], 8)
            raise _PhaseDone()
        # ---------------- pt convs ----------------
        def conv2x2(hin, wT, bias, outtag):
            outs = []
            for o in range(2):
                ps = psum(128, 256)
                nc.tensor.matmul(ps[:], wT[:, 0, o * 128:(o + 1) * 128], hin[0][:],
                                 start=True, stop=False)
                nc.tensor.matmul(ps[:], wT[:, 1, o * 128:(o + 1) * 128], hin[1][:],
                                 start=False, stop=True)
                ho = sp.tile([128, 256], F32R, tag=f'{outtag}_{o}')
                nc.scalar.activation(ho[:], ps[:], AF.Relu, bias=bias[:, o:o + 1])
                outs.append(ho)
            return outs

        hA = conv2x2(f1, ws['ptc1T'], ws['ptb1'], 'hA')
        h = conv2x2(hA, ws['ptc2T'], ws['ptb2'], 'hB')

        # ---------------- 4 offset-attention layers ----------------
        conc = []  # 10 [128, 256] k-chunks for the final conv, in order
        for i in range(4):
            wqkT, wvT = ws[f'sa{i}_wqkT'], ws[f'sa{i}_wvT']
            wtT, bfin = ws[f'sa{i}_wtT'], ws[f'sa{i}_bfin']
            # k/q: [64, 256]
            psk = psum(64, 256)
            nc.tensor.matmul(psk[:], wqkT[:, 0, :], h[0][:], start=True, stop=False)
            nc.tensor.matmul(psk[:], wqkT[:, 1, :], h[1][:], start=False, stop=True)
            kq = sp.tile([64, 256], F32R, tag='kq')
            nc.vector.tensor_copy(kq[:], psk[:])
            # energy + softmax per n-chunk
            attn = []
            rms = []
            for n in range(2):
                psE = psum(128, 256)
                nc.tensor.matmul(psE[:], kq[:, n * 128:(n + 1) * 128], kq[:],
                                 start=True, stop=True)
                mxn = sp.tile([128, 1], F32, tag='mxn')
                nc.vector.tensor_reduce(mxn[:], psE[:], AX, MAX, negate=True)
                aun = sp.tile([128, 256], F32, tag='aun')
                rs = sp.tile([128, 1], F32, tag='rs')
                nc.scalar.activation(aun[:], psE[:], AF.Exp, bias=mxn[:],
                                     accum_out=rs[:])
                rr = sp.tile([128, 1], F32, tag='rr')
                nc.vector.reciprocal(rr[:], rs[:])
                at = sp.tile([128, 256], F32R, tag=f'attn_{n}')
                nc.vector.tensor_scalar(at[:], aun[:], rr[:], None, MULT)
                attn.append(at)
            # column normalizer r = 1/(1e-9 + colsum), replicated to all rows
            rrow = sp.tile([1, 256], F32R, tag='rrow')
            for m in range(2):
                psc = psum(128, 4)
                nc.tensor.matmul(psc[:], attn[0][:, m * 128:(m + 1) * 128],
                                 ws['ones128col'][:], start=True, stop=False)
                nc.tensor.matmul(psc[:], attn[1][:, m * 128:(m + 1) * 128],
                                 ws['ones128col'][:], start=False, stop=True)
                cadd = sp.tile([128, 1], F32, tag='cadd')
                nc.vector.tensor_scalar(cadd[:], psc[:, 0:1], 1e-9, None, ADD)
                rm = sp.tile([128, 1], F32R, tag='rm')
                with nc.allow_low_precision('f32r bits == f32 bits here'):
                    nc.vector.reciprocal(rm[:], cadd[:])
                rms.append(rm)
                psr = psum(1, 128, F32R)
                nc.tensor.transpose(psr[:], rm[:], ws['id128'][:])
                nc.vector.tensor_copy(rrow[:, m * 128:(m + 1) * 128], psr[:])
            psrep = psum(128, 256)
            nc.tensor.matmul(psrep[:], ws['ones1'][:], rrow[:], start=True, stop=True)
            attn2 = []
            for n in range(2):
                a2 = sp.tile([128, 256], F32R, tag=f'attn2_{n}')
                nc.vector.tensor_tensor(out=a2[:], in0=attn[n][:], in1=psrep[:],
                                        op=MULT)
                attn2.append(a2)
            # vT [n, c] with bias seed
            vT = []
            for n in range(2):
                ps = psum(128, 256)
                nc.tensor.matmul(ps[:], ws['ones1'][:], ws[f'sa{i}_bvrow'][:],
                                 start=True, stop=False)
                nc.tensor.matmul(ps[:], h[0][:, n * 128:(n + 1) * 128], wvT[:, 0, :],
                                 start=False, stop=False)
                nc.tensor.matmul(ps[:], h[1][:, n * 128:(n + 1) * 128], wvT[:, 1, :],
                                 start=False, stop=True)
                vt = sp.tile([128, 256], F32R, tag=f'vT_{n}')
                nc.vector.tensor_copy(vt[:], ps[:])
                vT.append(vt)
            # x_r and d = h - x_r
            d = []
            for cc in range(2):
                ps = psum(128, 256)
                nc.tensor.matmul(ps[:], vT[0][:, cc * 128:(cc + 1) * 128],
                                 attn2[0][:], start=True, stop=False)
                nc.tensor.matmul(ps[:], vT[1][:, cc * 128:(cc + 1) * 128],
                                 attn2[1][:], start=False, stop=True)
                dc = sp.tile([128, 256], F32R, tag=f'd_{cc}')
                nc.vector.tensor_tensor(out=dc[:], in0=h[cc][:], in1=ps[:], op=SUB)
                d.append(dc)
            # t = relu(wts @ d + bfin); h = h + t
            hn = []
            for o in range(2):
                ps = psum(128, 256)
                nc.tensor.matmul(ps[:], wtT[:, 0, o * 128:(o + 1) * 128], d[0][:],
                                 start=True, stop=False)
                nc.tensor.matmul(ps[:], wtT[:, 1, o * 128:(o + 1) * 128], d[1][:],
                                 start=False, stop=True)
                xr2 = sp.tile([128, 256], F32, tag='xr2')
                nc.scalar.activation(xr2[:], ps[:], AF.Relu, bias=bfin[:, o:o + 1])
                hno = sp.tile([128, 256], F32R, tag=f'sa{i}o{o}')
                nc.vector.tensor_tensor(out=hno[:], in0=h[o][:], in1=xr2[:], op=ADD)
                hn.append(hno)
            h = hn
            conc.extend(hn)
        conc.extend(f1)

        if phase == 'att':
            _dbg_out(h[0][:, 0:8], 8)
            raise _PhaseDone()
        # ---------------- final conv + leaky relu + global max ----------------
        outsb = sp.tile([128, 8], F32, tag='outsb')
        cfT = ws['cfT'][:].rearrange('p (k n) -> p k n', k=10)
        for m in range(8):
            ps = psum(128, 256)
            for k in range(10):
                nc.tensor.matmul(ps[:], cfT[:, k, m * 128:(m + 1) * 128],
                                 conc[k][:], start=(k == 0), stop=(k == 9))
            mx = sp.tile([128, 1], F32, tag='mx')
            nc.vector.tensor_reduce(mx[:], ps[:], AX, MAX)
            y = sp.tile([128, 1], F32, tag='y')
            nc.vector.tensor_scalar(y[:], mx[:], ws['cfb'][:, m:m + 1], None, ADD)
            nc.vector.scalar_tensor_tensor(outsb[:, m:m + 1], y[:], 0.2, y[:],
                                           MULT, MAX)
        nc.sync.dma_start(out[:].rearrange('(m p) -> p m', p=128), outsb[:])

      except _PhaseDone:
        pass
    nc.compile()
    return nc


# ======================================================================
# Entry point
# ======================================================================

def kernel(x, params):
    from concourse.bass_utils import run_bass_kernel_spmd

    x = np.asarray(x, np.float32)
    B = x.shape[0]
    assert B == 8

    if 'nc' not in _CACHE:
        _CACHE['nc'] = build_nc()
    nc = _CACHE['nc']

    wmap = _prep_weights(params)
    in_maps = []
    for b in range(B):
        m = dict(wmap)
        m['x'] = np.ascontiguousarray(x[b])
        m.update(_host_geometry(x[b]))
        in_maps.append({k: np.ascontiguousarray(np.asarray(v)) for k, v in m.items()})

    res = run_bass_kernel_spmd(nc, in_maps, list(range(B)))
    return np.stack([res.results[b]['out'] for b in range(B)]).astype(np.float32)
